# revision 1
# baseline (speedup 1.0000x reference)
"""Binarized 3x3 conv + batchnorm(train) + sign, on 8 TRN2 NeuronCores.

Math: out = sign((y - mean)/sqrt(var+eps)) where y = conv(x, sign(w)) + sign(b)
and mean/var are per-channel batch stats.  Since sqrt(var+eps) > 0, the output
is exactly sign(y - mean_c): variance never needs to be computed.

Strategy (data-parallel over batch, 4 images/core):
 - implicit GEMM: for each of 9 (kh,kw) shifts and 2 input-channel blocks,
   accumulate W[128ci,128co].T @ x_shifted[128ci, 504px] into PSUM.
   Rows are packed 9-at-a-time (9*56=504 free dim); the 2 wrap-around columns
   per row are discarded at PSUM->SBUF drain time.
 - fp32-quality precision from a split matmul: x = hi + lo with hi = fp16(x)
   (full-rate on the PE) and lo = (x - hi)*4096 in fp8-e4m3 run in DoubleRow
   perf mode (half-cycle per row, both ci blocks folded into one matmul).
   Weights are exactly +-1 in fp16/fp8.  The drain combines
   y = hi_psum + lo_psum/4096 + bias and harvests per-channel sums for free
   (ScalarE activation / DVE tensor_tensor_reduce accumulators).
 - one tiny AllReduce (128x2 fp32) across the 8 cores for the global mean.
 - pass 2: t=(y+(-mean))>=0 on VectorE, out=2t-1 on ScalarE, DMA out as bf16
   (+-1 exact), host converts to fp32.
"""

import sys

if "/opt/trn_rl_repo" not in sys.path:
    sys.path.insert(0, "/opt/trn_rl_repo")

import numpy as np
import ml_dtypes

N_CORES = 8
N_PER_CORE = 4          # images per core
CI = 256                # in channels
CO = 256                # out channels
H = W = 56
OH = OW = 54
HWF = H * W             # 3136
HWPAD_H = HWF + 4       # fp16 hi image length in SBUF (pad for row overhang)
HWPAD_L = HWF + 16      # fp8 lo image length; pair-dim stride must be 16B-aligned
NPIX = OH * OW          # 2916
RT = 6                  # row tiles per image (9 rows each)
RROWS = 9
FREE = RROWS * W        # 504 matmul free dim
TFREE = RROWS * OW      # 486 valid outputs per tile
N_TOT = N_CORES * N_PER_CORE
MEAN_SCALE = 1.0 / (N_TOT * NPIX)
LO_SCALE = 4096.0       # lo stored as (x - fp16(x)) * LO_SCALE in fp8 e4m3
RTG = 3                 # rt tiles per psum group (3 hi + 3 lo banks)

BF16 = ml_dtypes.bfloat16
FP8 = ml_dtypes.float8_e4m3


def build(nc, n_cores=N_CORES):
    """Emit the SPMD program into a bacc.Bacc instance."""
    import concourse.mybir as mybir
    from concourse import tile

    f32 = mybir.dt.float32
    f16 = mybir.dt.float16
    bf16 = mybir.dt.bfloat16
    fp8 = mybir.dt.float8e4
    ACT = mybir.ActivationFunctionType
    DR = mybir.MatmulPerfMode.DoubleRow

    xh_d = nc.dram_tensor("xh", [N_PER_CORE, 128, 2, HWPAD_H], f16, kind="ExternalInput")
    xl_d = nc.dram_tensor("xl", [N_PER_CORE, 128, 2, HWPAD_L], fp8, kind="ExternalInput")
    w_d = nc.dram_tensor("wt", [128, 2 * 2 * 9 * 128], f16, kind="ExternalInput")
    w8_d = nc.dram_tensor("w8", [128, 2 * 9 * 2 * 128], fp8, kind="ExternalInput")
    y_d = nc.dram_tensor("y", [N_PER_CORE, 2, 128, NPIX], mybir.dt.uint8, kind="ExternalOutput")

    n_tiles = N_PER_CORE * 2 * RT  # 48

    with tile.TileContext(nc) as tc:
        with (
            tc.tile_pool(name="wpool", bufs=1) as wpool,
            tc.tile_pool(name="xpool", bufs=2) as xpool,
            tc.tile_pool(name="ypool", bufs=1) as ypool,
            tc.tile_pool(name="spool", bufs=1) as spool,
            tc.tile_pool(name="opool", bufs=4) as opool,
            tc.tile_pool(name="tpool", bufs=6) as tpool,
            tc.tile_pool(name="pspool", bufs=8, space="PSUM") as pspool,
            tc.tile_pool(name="drampool", bufs=2, space="DRAM") as drampool,
        ):
            w_sb = wpool.tile([128, 2 * 2 * 9 * 128], f16)
            nc.sync.dma_start(w_sb[:], w_d[:])
            w8_sb = wpool.tile([128, 2, 9, 2, 128], fp8)
            nc.sync.dma_start(
                w8_sb[:],
                w8_d[:].rearrange("p (c s b m) -> p c s b m", c=2, s=9, b=2),
            )
            y_sb = ypool.tile([128, n_tiles * TFREE], f32)
            sums = spool.tile([128, n_tiles], f32, tag="sums")

            # ---------------- phase 1: conv + drain (+bias, +sums) ----------
            for n in range(N_PER_CORE):
                xh_sb = xpool.tile([128, 2, HWPAD_H], f16, tag="xh")
                xl_sb = xpool.tile([128, 2, HWPAD_L], fp8, tag="xl")
                nc.sync.dma_start(xh_sb[:], xh_d[n])
                nc.sync.dma_start(xl_sb[:], xl_d[n])

                for cb in range(2):
                    for rtg in range(RT // RTG):
                        rts = [rtg * RTG + i for i in range(RTG)]
                        hps = [
                            pspool.tile([128, TFREE], f32, tag="ps", name=f"hps{i}")
                            for i in range(RTG)
                        ]
                        lps = [
                            pspool.tile([128, FREE], f32, tag="ps", name=f"lps{i}")
                            for i in range(RTG)
                        ]
                        # lo pass first: fp8 DoubleRow, both ci blocks per
                        # matmul, s-outer so each DR weight load feeds RTG MMs
                        for s in range(9):
                            kh, kw = divmod(s, 3)
                            lw8 = w8_sb[:, cb, s]
                            for i, rt in enumerate(rts):
                                off = (rt * RROWS + kh) * W + kw
                                nc.tensor.matmul(
                                    lps[i][:],
                                    lw8,
                                    xl_sb[:, :, off : off + FREE],
                                    start=(s == 0),
                                    stop=(s == 8),
                                    perf_mode=DR,
                                )
                        # lo drains can run on ScalarE while the hi pass
                        # computes, freeing the lo banks early
                        tmps = []
                        for i, rt in enumerate(rts):
                            tmp = tpool.tile([128, TFREE], f32, tag="tmp", name=f"tmp{i}")
                            lps_v = lps[i][:].rearrange("p (r c) -> p r c", c=W)[
                                :, :, 0:OW
                            ]
                            nc.scalar.activation(
                                tmp[:].rearrange("p (r c) -> p r c", c=OW),
                                lps_v,
                                ACT.Copy,
                                scale=1.0 / LO_SCALE,
                            )
                            tmps.append(tmp)
                        # hi pass: fp16, rt-outer (FWL makes the extra weight
                        # loads ~free) so tile i's drain overlaps tile i+1's MMs
                        for i, rt in enumerate(rts):
                            for b in range(2):
                                for s in range(9):
                                    kh, kw = divmod(s, 3)
                                    k = ((b * 2 + cb) * 9 + s) * 128
                                    lw = w_sb[:, k : k + 128]
                                    first = b == 0 and s == 0
                                    last = b == 1 and s == 8
                                    off = (rt * RROWS + kh) * W + kw
                                    rhs = xh_sb[:, b, off : off + FREE].rearrange(
                                        "p (r c) -> p r c", c=W
                                    )[:, :, 0:OW]
                                    nc.tensor.matmul(
                                        hps[i][:],
                                        lw,
                                        rhs,
                                        start=first,
                                        stop=last,
                                    )
                            # drain: y = hi + lo/LO_SCALE ; accum channel sums
                            # (the +-1 channel bias cancels exactly in
                            # sign(y-mean), so it is dropped)
                            t = (cb * N_PER_CORE + n) * RT + rt
                            nc.vector.scalar_tensor_tensor(
                                y_sb[:, t * TFREE : (t + 1) * TFREE],
                                hps[i][:],
                                1.0,
                                tmps[i][:],
                                mybir.AluOpType.mult,
                                mybir.AluOpType.add,
                                accum_out=sums[:, t : t + 1],
                            )

            # ---------------- phase 2: global mean via AllReduce ------------
            sums2 = spool.tile([128, 2], f32, tag="sums2")
            # tile index t = (cb*N + n)*RT + rt, so cb is outermost:
            # one X-axis reduce over the 24 per-cb columns.
            nc.vector.tensor_reduce(
                sums2[:],
                sums[:].rearrange("p (c m) -> p c m", c=2),
                axis=mybir.AxisListType.X,
                op=mybir.AluOpType.add,
            )
            sums_g = spool.tile([128, 2], f32, tag="sumsg")
            if n_cores > 1:
                cc_in = drampool.tile([128, 2], f32)
                cc_out = drampool.tile([128, 2], f32)
                nc.sync.dma_start(cc_in[:], sums2[:])
                nc.gpsimd.collective_compute(
                    "AllReduce",
                    mybir.AluOpType.add,
                    replica_groups=[list(range(n_cores))],
                    ins=[cc_in.opt()],
                    outs=[cc_out.opt()],
                )
                nc.sync.dma_start(sums_g[:], cc_out[:])
            else:
                # single-core timing variant (TimelineSim can't model
                # collectives): mean is just this core's sums
                nc.vector.tensor_copy(sums_g[:], sums2[:])
            neg_mean = spool.tile([128, 2], f32, tag="negmean")
            nc.scalar.mul(neg_mean[:], sums_g[:], -MEAN_SCALE)

            # ---------------- phase 3: binarize + store ---------------------
            # bin = (y + (-mean)) >= 0 in {0,1} bf16; host maps to +-1 fp32.
            # One DVE op + one DMA per (cb, n) image-block (6 rt tiles = the
            # whole contiguous [128, 2916] slab).
            # bin = (y + (-mean)) >= 0 in {0,1} uint8 (1B/elem halves the
            # output DMA); host maps to +-1 fp32.
            for cb in range(2):
                for n in range(N_PER_CORE):
                    t0 = (cb * N_PER_CORE + n) * RT
                    bin_t = opool.tile([128, RT * TFREE], mybir.dt.uint8, tag="bin")
                    nc.vector.tensor_scalar(
                        bin_t[:],
                        y_sb[:, t0 * TFREE : (t0 + RT) * TFREE],
                        neg_mean[:, cb : cb + 1],
                        0.0,
                        mybir.AluOpType.add,
                        mybir.AluOpType.is_ge,
                    )
                    nc.sync.dma_start(y_d[n, cb], bin_t[:])

    nc.compile()
    return nc


def prep_inputs(x, weight, bias):
    """Host-side shard + layout prep. Returns list of 8 per-core input maps."""
    assert x.shape == (N_TOT, CI, H, W) and x.dtype == np.float32

    # x -> [core, n, p, b, hw]; hi = fp16(x), lo = (x - hi)*LO_SCALE in fp8
    xs = x.reshape(N_CORES, N_PER_CORE, 2, 128, HWF).transpose(0, 1, 3, 2, 4)
    xh = xs.astype(np.float16)
    xl = ((xs - xh.astype(np.float32)) * np.float32(LO_SCALE)).astype(FP8)
    xh = np.pad(xh, ((0, 0),) * 4 + ((0, HWPAD_H - HWF),))
    xl = np.pad(xl, ((0, 0),) * 4 + ((0, HWPAD_L - HWF),))

    wb = np.where(weight >= 0, np.float32(1.0), np.float32(-1.0))
    # [cb, co_f, b, ci_p, kh, kw] -> hi: [ci_p, b, cb, (kh kw), co_f]
    w6 = wb.reshape(2, 128, 2, 128, 3, 3)
    wt = (
        w6.transpose(3, 2, 0, 4, 5, 1)
        .reshape(128, 2 * 2 * 9 * 128)
        .astype(np.float16)
    )
    # lo: [ci_p, cb, (kh kw), b, co_f]
    w8 = (
        w6.transpose(3, 0, 4, 5, 2, 1)
        .reshape(128, 2 * 9 * 2 * 128)
        .astype(FP8)
    )
    return [
        {
            "xh": np.ascontiguousarray(xh[c]),
            "xl": np.ascontiguousarray(xl[c]),
            "wt": wt,
            "w8": w8,
        }
        for c in range(N_CORES)
    ]


def gather(results):
    """[{y: [4,2,128,2916] uint8 in {0,1}}] * 8 -> (32, 256, 54, 54) fp32 +-1."""
    ys = np.stack([r["y"] for r in results])
    out = ys.astype(np.float32).reshape(N_TOT, CO, OH, OW)
    return out * np.float32(2.0) - np.float32(1.0)


_STATE = {}


def _get_nc():
    if "nc" not in _STATE:
        import concourse.bacc as bacc

        nc = bacc.Bacc(
            "TRN2", target_bir_lowering=False, debug=False, num_devices=N_CORES
        )
        _STATE["nc"] = build(nc)
    return _STATE["nc"]


def kernel(x, weight, bias, _trace=False):
    from concourse.bass_utils import run_bass_kernel_spmd

    nc = _get_nc()
    in_maps = prep_inputs(
        np.asarray(x, np.float32),
        np.asarray(weight, np.float32),
        np.asarray(bias, np.float32),
    )
    res = run_bass_kernel_spmd(
        nc, in_maps, core_ids=list(range(N_CORES)), trace=_trace
    )
    _STATE["last_result"] = res
    return gather(res.results)



# revision 3
# speedup vs baseline: 1.5828x; 1.5828x over previous
"""Binarized 3x3 conv + batchnorm(train) + sign, on 8 TRN2 NeuronCores.

Math: out = sign((y - mean)/sqrt(var+eps)) where y = conv(x, sign(w)) + sign(b)
and mean/var are per-channel batch stats.  Since sqrt(var+eps) > 0, the output
is exactly sign(y - mean_c): variance never needs to be computed.  The +-1
channel bias cancels exactly in sign(y - mean), so it is dropped.

Strategy (data-parallel over batch, 4 images/core):
 - implicit GEMM, all-fp8 DoubleRow: x is split into three e4m3 planes
   x ~= x0 + x1/64 + x2/64 with x0 = e4m3(x), x1 = e4m3((x-x0)*64),
   x2 = e4m3((x-x0-x1/64)*64).  Weights are +-1 (exact in fp8); levels 1,2
   use weights pre-scaled by 2^-6 (exact fp8 normal), so all three levels
   accumulate into a single fp32 PSUM bank: 3 levels x 9 shifts = 27
   DoubleRow matmuls (each contracting both 128-channel input blocks at
   half-cycle/row) per output tile.  ~2x fewer PE cycles than an
   fp16-hi + fp8-lo split, at ~130/23.9M sign flips.
 - rhs of each matmul is a 4D view [128ci, 2b, 9rows, 54cols] so only valid
   output columns are computed (486 of 504).
 - drain PSUM -> y_sb (fp32) alternates DVE / ScalarE, harvesting per-channel
   sums via accum_out for free.
 - one tiny AllReduce (128x2 fp32) across the 8 cores for the global mean.
 - binarize t = (y + (-mean)) >= 0 split across DVE / ScalarE / Pool engines;
   DVE+Pool emit {0,1} uint8, ScalarE emits Sign() as fp8 (+-1/0); host maps
   both encodings to +-1 fp32.
"""

import sys

if "/opt/trn_rl_repo" not in sys.path:
    sys.path.insert(0, "/opt/trn_rl_repo")

import numpy as np
import ml_dtypes

N_CORES = 8
N_PER_CORE = 4          # images per core
CI = 256                # in channels
CO = 256                # out channels
H = W = 56
OH = OW = 54
HWF = H * W             # 3136
HWPAD = HWF + 16        # fp8 image length; pair-dim stride must be 16B-aligned
NPIX = OH * OW          # 2916
RT = 6                  # row tiles per image (9 rows each)
RROWS = 9
FREE = RROWS * W        # 504 slice length per shift
TFREE = RROWS * OW      # 486 valid outputs per tile
N_TOT = N_CORES * N_PER_CORE
MEAN_SCALE = 1.0 / (N_TOT * NPIX)
NLVL = 3                # fp8 residual levels
LVL_SCALE = 64.0        # per-level residual scale (weights get 1/64, exact fp8)
RTG = 3                 # rt tiles per psum group

FP8 = ml_dtypes.float8_e4m3

# phase-3 engine split: (cb, n) image blocks, cb-major order (8 blocks)
ACT_BLOCKS = (0, 1, 2)          # ScalarE: Sign() -> fp8 +-1
DVE_BLOCKS = (3, 4, 5)          # DVE: is_ge -> uint8 {0,1}
POOL_BLOCKS = (6, 7)            # Pool: is_ge -> uint8 {0,1}


def build(nc, n_cores=N_CORES):
    """Emit the SPMD program into a bacc.Bacc instance."""
    import concourse.mybir as mybir
    from concourse import tile

    f32 = mybir.dt.float32
    fp8 = mybir.dt.float8e4
    ACT = mybir.ActivationFunctionType
    DR = mybir.MatmulPerfMode.DoubleRow

    x_d = nc.dram_tensor("x8", [N_PER_CORE, NLVL, 128, 2, HWPAD], fp8,
                         kind="ExternalInput")
    w_d = nc.dram_tensor("w8", [128, 2 * 9 * 2 * 128], fp8, kind="ExternalInput")
    ws_d = nc.dram_tensor("w8s", [128, 2 * 9 * 2 * 128], fp8, kind="ExternalInput")
    y_d = nc.dram_tensor("y", [N_PER_CORE, 2, 128, NPIX], mybir.dt.uint8,
                         kind="ExternalOutput")
    ys_d = nc.dram_tensor("ys", [N_PER_CORE, 2, 128, NPIX], fp8,
                          kind="ExternalOutput")

    n_tiles = N_PER_CORE * 2 * RT  # 48

    with tile.TileContext(nc) as tc:
        with (
            tc.tile_pool(name="wpool", bufs=1) as wpool,
            tc.tile_pool(name="xpool", bufs=2) as xpool,
            tc.tile_pool(name="ypool", bufs=1) as ypool,
            tc.tile_pool(name="spool", bufs=1) as spool,
            tc.tile_pool(name="opool", bufs=4) as opool,
            tc.tile_pool(name="pspool", bufs=6, space="PSUM") as pspool,
            tc.tile_pool(name="drampool", bufs=2, space="DRAM") as drampool,
        ):
            w_sb = wpool.tile([128, 2, 9, 2, 128], fp8, name="w8")
            nc.sync.dma_start(
                w_sb[:], w_d[:].rearrange("p (c s b m) -> p c s b m", c=2, s=9, b=2)
            )
            ws_sb = wpool.tile([128, 2, 9, 2, 128], fp8, name="w8s")
            nc.sync.dma_start(
                ws_sb[:], ws_d[:].rearrange("p (c s b m) -> p c s b m", c=2, s=9, b=2)
            )
            y_sb = ypool.tile([128, n_tiles * TFREE], f32)
            sums = spool.tile([128, n_tiles], f32, tag="sums")

            # ---------------- phase 1: conv + drain (+sums) ------------------
            drain_eng = [nc.vector, nc.scalar]
            for n in range(N_PER_CORE):
                x_sb = [
                    xpool.tile([128, 2, HWPAD], fp8, tag=f"x{l}", name=f"x{l}")
                    for l in range(NLVL)
                ]
                for l in range(NLVL):
                    nc.sync.dma_start(x_sb[l][:], x_d[n, l])

                for cb in range(2):
                    for rtg in range(RT // RTG):
                        rts = [rtg * RTG + i for i in range(RTG)]
                        pss = [
                            pspool.tile([128, TFREE], f32, tag="ps", name=f"ps{i}")
                            for i in range(RTG)
                        ]
                        for l in range(NLVL):
                            w_l = w_sb if l == 0 else ws_sb
                            for s in range(9):
                                kh, kw = divmod(s, 3)
                                lw = w_l[:, cb, s]
                                for i, rt in enumerate(rts):
                                    off = (rt * RROWS + kh) * W + kw
                                    rhs = (
                                        x_sb[l][:, :, off : off + FREE]
                                        .rearrange("p b (r c) -> p b r c", c=W)
                                        [:, :, :, 0:OW]
                                    )
                                    nc.tensor.matmul(
                                        pss[i][:],
                                        lw,
                                        rhs,
                                        start=(l == 0 and s == 0),
                                        stop=(l == NLVL - 1 and s == 8),
                                        perf_mode=DR,
                                    )
                        # drain: y = psum; accum channel sums for free
                        for i, rt in enumerate(rts):
                            t = (cb * N_PER_CORE + n) * RT + rt
                            eng = drain_eng[t % 2]
                            if eng is nc.scalar:
                                nc.scalar.activation(
                                    y_sb[:, t * TFREE : (t + 1) * TFREE],
                                    pss[i][:],
                                    ACT.Copy,
                                    accum_out=sums[:, t : t + 1],
                                )
                            else:
                                nc.vector.tensor_scalar(
                                    y_sb[:, t * TFREE : (t + 1) * TFREE],
                                    pss[i][:],
                                    1.0,
                                    0.0,
                                    mybir.AluOpType.mult,
                                    mybir.AluOpType.add,
                                    accum_out=sums[:, t : t + 1],
                                )

            # ---------------- phase 2: global mean via AllReduce ------------
            sums2 = spool.tile([128, 2], f32, tag="sums2")
            # tile index t = (cb*N + n)*RT + rt, so cb is outermost:
            # one X-axis reduce over the 24 per-cb columns.
            nc.vector.tensor_reduce(
                sums2[:],
                sums[:].rearrange("p (c m) -> p c m", c=2),
                axis=mybir.AxisListType.X,
                op=mybir.AluOpType.add,
            )
            sums_g = spool.tile([128, 2], f32, tag="sumsg")
            if n_cores > 1:
                cc_in = drampool.tile([128, 2], f32)
                cc_out = drampool.tile([128, 2], f32)
                nc.sync.dma_start(cc_in[:], sums2[:])
                nc.gpsimd.collective_compute(
                    "AllReduce",
                    mybir.AluOpType.add,
                    replica_groups=[list(range(n_cores))],
                    ins=[cc_in.opt()],
                    outs=[cc_out.opt()],
                )
                nc.sync.dma_start(sums_g[:], cc_out[:])
            else:
                # single-core timing variant (TimelineSim can't model
                # collectives): mean is just this core's sums
                nc.vector.tensor_copy(sums_g[:], sums2[:])
            neg_mean = spool.tile([128, 2], f32, tag="negmean")
            nc.scalar.mul(neg_mean[:], sums_g[:], -MEAN_SCALE)

            # ---------------- phase 3: binarize + store ---------------------
            # DVE/Pool: bin = (y + (-mean)) >= 0 in {0,1} uint8.
            # ScalarE:  bin = Sign(y + (-mean)) in {-1,0,+1} fp8.
            # Host maps both to +-1 fp32 (0 counts as >= 0).
            for cb in range(2):
                for n in range(N_PER_CORE):
                    blk = cb * N_PER_CORE + n
                    t0 = (cb * N_PER_CORE + n) * RT
                    ysl = y_sb[:, t0 * TFREE : (t0 + RT) * TFREE]
                    if blk in ACT_BLOCKS:
                        bin_t = opool.tile([128, RT * TFREE], fp8, tag="bin8")
                        nc.scalar.activation(
                            bin_t[:],
                            ysl,
                            ACT.Sign,
                            bias=neg_mean[:, cb : cb + 1],
                            scale=1.0,
                        )
                        nc.sync.dma_start(ys_d[n, cb], bin_t[:])
                    else:
                        eng = nc.vector if blk in DVE_BLOCKS else nc.gpsimd
                        bin_t = opool.tile(
                            [128, RT * TFREE], mybir.dt.uint8, tag="binu"
                        )
                        eng.tensor_scalar(
                            bin_t[:],
                            ysl,
                            neg_mean[:, cb : cb + 1],
                            0.0,
                            mybir.AluOpType.add,
                            mybir.AluOpType.is_ge,
                        )
                        nc.sync.dma_start(y_d[n, cb], bin_t[:])

    nc.compile()
    return nc


def prep_inputs(x, weight, bias):
    """Host-side shard + layout prep. Returns list of 8 per-core input maps."""
    assert x.shape == (N_TOT, CI, H, W) and x.dtype == np.float32

    # x -> [core, n, p, b, hw]; 3-level fp8 split
    xs = x.reshape(N_CORES, N_PER_CORE, 2, 128, HWF).transpose(0, 1, 3, 2, 4)
    x0 = xs.astype(FP8)
    r1 = xs - x0.astype(np.float32)
    x1 = (r1 * np.float32(LVL_SCALE)).astype(FP8)
    r2 = r1 - x1.astype(np.float32) / np.float32(LVL_SCALE)
    x2 = (r2 * np.float32(LVL_SCALE)).astype(FP8)
    x8 = np.stack([x0, x1, x2], axis=2)  # [core, n, lvl, p, b, hw]
    x8 = np.pad(x8, ((0, 0),) * 5 + ((0, HWPAD - HWF),))

    wb = np.where(weight >= 0, np.float32(1.0), np.float32(-1.0))
    # [cb, co_f, b, ci_p, kh, kw] -> [ci_p, cb, (kh kw), b, co_f]
    w6 = wb.reshape(2, 128, 2, 128, 3, 3)
    w8 = w6.transpose(3, 0, 4, 5, 2, 1).reshape(128, 2 * 9 * 2 * 128).astype(FP8)
    w8s = (
        (w6 * np.float32(1.0 / LVL_SCALE))
        .transpose(3, 0, 4, 5, 2, 1)
        .reshape(128, 2 * 9 * 2 * 128)
        .astype(FP8)
    )
    return [
        {
            "x8": np.ascontiguousarray(x8[c]),
            "w8": w8,
            "w8s": w8s,
        }
        for c in range(N_CORES)
    ]


def gather(results):
    """[{y: u8 {0,1}, ys: fp8 {-1,0,1}}] * 8 -> (32, 256, 54, 54) fp32 +-1."""
    out = np.empty((N_CORES, N_PER_CORE, 2, 128, NPIX), np.float32)
    for c, r in enumerate(results):
        yu = r["y"]                      # uint8 {0,1}
        yf = r["ys"].view(FP8).astype(np.float32)  # fp8 {-1,0,+1}
        for cb in range(2):
            for n in range(N_PER_CORE):
                blk = cb * N_PER_CORE + n
                if blk in ACT_BLOCKS:
                    out[c, n, cb] = np.where(yf[n, cb] >= 0, 1.0, -1.0)
                else:
                    out[c, n, cb] = yu[n, cb].astype(np.float32) * 2.0 - 1.0
    # [core, n, cb, 128, pix] -> [N, 256, 54, 54]
    return out.reshape(N_TOT, CO, OH, OW)


_STATE = {}


def _get_nc():
    if "nc" not in _STATE:
        import concourse.bacc as bacc

        nc = bacc.Bacc(
            "TRN2", target_bir_lowering=False, debug=False, num_devices=N_CORES
        )
        _STATE["nc"] = build(nc)
    return _STATE["nc"]


def kernel(x, weight, bias, _trace=False):
    from concourse.bass_utils import run_bass_kernel_spmd

    nc = _get_nc()
    in_maps = prep_inputs(
        np.asarray(x, np.float32),
        np.asarray(weight, np.float32),
        np.asarray(bias, np.float32),
    )
    res = run_bass_kernel_spmd(
        nc, in_maps, core_ids=list(range(N_CORES)), trace=_trace
    )
    _STATE["last_result"] = res
    return gather(res.results)


# revision 25
# speedup vs baseline: 1.7166x; 1.0845x over previous
"""Binarized 3x3 conv + batchnorm(train) + sign, on 8 TRN2 NeuronCores.

Math: out = sign((y - mean)/sqrt(var+eps)) where y = conv(x, sign(w)) + sign(b)
and mean/var are per-channel batch stats.  Since sqrt(var+eps) > 0, the output
is exactly sign(y - mean_c): variance never needs to be computed.  The +-1
channel bias cancels exactly in sign(y - mean), so it is dropped.

Strategy (data-parallel over batch, 4 images/core):
 - implicit GEMM, all-fp8 DoubleRow: x is split into three e4m3 planes
   x ~= x0 + x1/64 + x2/64 with x0 = e4m3(x), x1 = e4m3((x-x0)*64),
   x2 = e4m3((x-x0-x1/64)*64).  Weights are +-1 (exact in fp8); levels 1,2
   use weights pre-scaled by 2^-6 (exact fp8 normal), so all three levels
   accumulate into a single fp32 PSUM bank: 3 levels x 9 shifts = 27
   DoubleRow matmuls (each contracting both 128-channel input blocks at
   half-cycle/row) per output tile.  ~2x fewer PE cycles than an
   fp16-hi + fp8-lo split, at ~130/23.9M sign flips.
 - rhs of each matmul is a 4D view [128ci, 2b, 9rows, 54cols] so only valid
   output columns are computed (486 of 504).
 - a ~140-matmul warmup chain on scratch data runs during the input-DMA
   lead-in so the PE p-state ramp completes before real matmuls start;
   lead-in DMAs are split/ordered so the first tile's weights + rows arrive
   first (DMA transfers serialize on the DMA engines).
 - drain PSUM -> y_sb (fp32) alternates DVE / ScalarE, harvesting per-channel
   sums via accum_out for free.
 - batch stats are done per output-channel-block (cb): cb=0's sums finish one
   image-pass (~16us) before cb=1's, so cb=0's AllReduce + binarize + store
   hide entirely under cb=1's conv matmuls; only cb=1's tail is exposed.
 - binarize splits each image's columns across ScalarE (Sign -> fp8 +-1/0),
   DVE and Pool (is_ge -> fp8 {0,1}); host maps both encodings to +-1 fp32.
"""

import sys

if "/opt/trn_rl_repo" not in sys.path:
    sys.path.insert(0, "/opt/trn_rl_repo")

import numpy as np
import ml_dtypes

N_CORES = 8
N_PER_CORE = 4          # images per core
CI = 256                # in channels
CO = 256                # out channels
H = W = 56
OH = OW = 54
HWF = H * W             # 3136
HWPAD = HWF + 16        # fp8 image length; pair-dim stride must be 16B-aligned
NPIX = OH * OW          # 2916
RT = 6                  # row tiles per image (9 rows each)
RROWS = 9
FREE = RROWS * W        # 504 slice length per shift
TFREE = RROWS * OW      # 486 valid outputs per tile
N_TOT = N_CORES * N_PER_CORE
MEAN_SCALE = 1.0 / (N_TOT * NPIX)
NLVL = 3                # fp8 residual levels
LVL_SCALE = 64.0        # per-level residual scale (weights get 1/64, exact fp8)
RTG = 3                 # rt tiles per psum group
NWUP = 160              # p-state warmup matmuls
XC1 = 1632              # image-0 chunk split: chunk1 [0:XC1) covers rtg 0
ACOLS = 815             # binarize column split: ScalarE share
DCOLS = 1546            # DVE share (runs 2x on SBUF operands); Pool gets 555

FP8 = ml_dtypes.float8_e4m3


def build(nc, n_cores=N_CORES, stop_after=None):
    """Emit the SPMD program into a bacc.Bacc instance."""
    import concourse.mybir as mybir
    from concourse import tile

    f32 = mybir.dt.float32
    fp8 = mybir.dt.float8e4
    ACT = mybir.ActivationFunctionType
    DR = mybir.MatmulPerfMode.DoubleRow

    x_d = nc.dram_tensor("x8", [N_PER_CORE, NLVL, 128, 2, HWPAD], fp8,
                         kind="ExternalInput")
    w_d = nc.dram_tensor("w8", [128, 2, 9 * 2 * 128], fp8, kind="ExternalInput")
    ws_d = nc.dram_tensor("w8s", [128, 2, 9 * 2 * 128], fp8, kind="ExternalInput")
    y_d = nc.dram_tensor("y8", [N_PER_CORE, 2, 128, NPIX], fp8,
                         kind="ExternalOutput")

    n_tiles = N_PER_CORE * 2 * RT  # 48

    with tile.TileContext(nc) as tc:
        with (
            tc.tile_pool(name="wpool", bufs=1) as wpool,
            tc.tile_pool(name="xpool", bufs=2) as xpool,
            tc.tile_pool(name="ypool", bufs=1) as ypool,
            tc.tile_pool(name="spool", bufs=1) as spool,
            tc.tile_pool(name="opool", bufs=4) as opool,
            tc.tile_pool(name="pspool", bufs=6, space="PSUM") as pspool,
            tc.tile_pool(name="wups", bufs=1, space="PSUM") as wups,
            tc.tile_pool(name="drampool", bufs=4, space="DRAM") as drampool,
        ):
            # ---------------- p-state warmup + act-table preload -------------
            wu_w = wpool.tile([128, 2, 128], fp8, name="wu")
            nc.vector.memset(wu_w[:], 1.0)
            scr_in = spool.tile([128, 1], f32, tag="scr0")
            scr_out = spool.tile([128, 1], fp8, tag="scr1")
            nc.vector.memset(scr_in[:], 0.0)
            nc.scalar.activation(scr_out[:], scr_in[:], ACT.Sign)
            wu_ps = wups.tile([128, 128], f32)
            for _ in range(NWUP):
                nc.tensor.matmul(
                    wu_ps[:], wu_w[:], wu_w[:], start=True, stop=True,
                    perf_mode=DR,
                )

            # ---------------- weights + image-0 lead-in DMAs ----------------
            # one serial DMA pipe: order so the first matmuls' deps land first
            w_sb = wpool.tile([128, 2, 9, 2, 128], fp8, name="w8")
            ws_sb = wpool.tile([128, 2, 9, 2, 128], fp8, name="w8s")
            wv = w_d[:].rearrange("p c (s b m) -> p c s b m", s=9, b=2)
            wsv = ws_d[:].rearrange("p c (s b m) -> p c s b m", s=9, b=2)
            x0_sb = [
                xpool.tile([128, 2, HWPAD], fp8, tag=f"x{l}", name=f"x{l}")
                for l in range(NLVL)
            ]
            # deps are tracked as flat byte intervals: a matmul's rhs read
            # [off:off+FREE] of both b blocks flattens to one interval
            # covering all of b0 and b1's head, so only b1's tail may arrive
            # late.  Per level: b0 full + b1 head early, b1 tail later.
            def xb0(l):
                nc.sync.dma_start(x0_sb[l][:, 0], x_d[0, l, :, 0])

            def xb1(l, head):
                sl = slice(0, XC1) if head else slice(XC1, HWPAD)
                nc.sync.dma_start(x0_sb[l][:, 1, sl], x_d[0, l, :, 1, sl])

            nc.sync.dma_start(w_sb[:, 0], wv[:, 0])
            xb0(0)
            xb1(0, True)
            nc.sync.dma_start(ws_sb[:, 0], wsv[:, 0])
            xb0(1)
            xb1(1, True)
            xb0(2)
            xb1(2, True)
            xb1(0, False)
            xb1(1, False)
            nc.sync.dma_start(w_sb[:, 1], wv[:, 1])
            xb1(2, False)
            nc.sync.dma_start(ws_sb[:, 1], wsv[:, 1])

            y_sb = ypool.tile([128, n_tiles * TFREE], f32)
            sums = spool.tile([128, n_tiles], f32, tag="sums")

            # ---------------- phase 1: conv + drain (+sums) ------------------
            for n in range(N_PER_CORE):
                if n == 0:
                    x_sb = x0_sb
                else:
                    x_sb = [
                        xpool.tile([128, 2, HWPAD], fp8, tag=f"x{l}", name=f"x{l}")
                        for l in range(NLVL)
                    ]
                    for l in range(NLVL):
                        nc.sync.dma_start(x_sb[l][:], x_d[n, l])

                for cb in range(2):
                    # last (n, cb)'s groups are [4, 2] tiles so the final
                    # group's two drains run in parallel on DVE + ScalarE
                    groups = (
                        ([0, 1, 2, 3], [4, 5])
                        if (n == N_PER_CORE - 1 and cb == 1)
                        else ([0, 1, 2], [3, 4, 5])
                    )
                    for rts in groups:
                        pss = [
                            pspool.tile([128, TFREE], f32, tag="ps", name=f"ps{i}")
                            for i in range(len(rts))
                        ]
                        for l in range(NLVL):
                            w_l = w_sb if l == 0 else ws_sb
                            for s in range(9):
                                kh, kw = divmod(s, 3)
                                lw = w_l[:, cb, s]
                                for i, rt in enumerate(rts):
                                    off = (rt * RROWS + kh) * W + kw
                                    rhs = (
                                        x_sb[l][:, :, off : off + FREE]
                                        .rearrange("p b (r c) -> p b r c", c=W)
                                        [:, :, :, 0:OW]
                                    )
                                    nc.tensor.matmul(
                                        pss[i][:],
                                        lw,
                                        rhs,
                                        start=(l == 0 and s == 0),
                                        stop=(l == NLVL - 1 and s == 8),
                                        perf_mode=DR,
                                    )
                        # drain: y = psum; accum channel sums for free
                        # (Pool/GPSIMD cannot read PSUM, so DVE/ScalarE only)
                        for i, rt in enumerate(rts):
                            t = (cb * N_PER_CORE + n) * RT + rt
                            if t % 2 == 0:
                                nc.scalar.activation(
                                    y_sb[:, t * TFREE : (t + 1) * TFREE],
                                    pss[i][:],
                                    ACT.Copy,
                                    accum_out=sums[:, t : t + 1],
                                )
                            else:
                                nc.vector.tensor_scalar(
                                    y_sb[:, t * TFREE : (t + 1) * TFREE],
                                    pss[i][:],
                                    1.0,
                                    0.0,
                                    mybir.AluOpType.mult,
                                    mybir.AluOpType.add,
                                    accum_out=sums[:, t : t + 1],
                                )

            # ------- phase 2+3 per cb: mean AllReduce, binarize, store -------
            # cb=0's sums are complete one image-pass (~16us) before cb=1's,
            # so its AllReduce + binarize + DMA hide under cb=1's matmuls.
            do_p2 = stop_after not in ("conv",)
            do_p3 = stop_after not in ("conv", "mean")
            for cb in range(2) if do_p2 else ():
                # fused reduce+scale: s2 = sum(-MEAN_SCALE * sums[cb]); the
                # AllReduce is linear so pre-scaling yields -mean directly.
                junk = spool.tile([128, 24], f32, tag=f"junk{cb}")
                s2 = spool.tile([128, 1], f32, tag=f"s2_{cb}")
                nc.vector.tensor_scalar(
                    junk[:],
                    sums[:, cb * 24 : (cb + 1) * 24],
                    -MEAN_SCALE,
                    0.0,
                    mybir.AluOpType.mult,
                    mybir.AluOpType.add,
                    accum_out=s2[:],
                )
                sg = spool.tile([128, 1], f32, tag=f"sg_{cb}")
                if n_cores > 1:
                    cc_in = drampool.tile([128, 1], f32)
                    cc_out = drampool.tile([128, 1], f32)
                    nc.sync.dma_start(cc_in[:], s2[:])
                    nc.gpsimd.collective_compute(
                        "AllReduce",
                        mybir.AluOpType.add,
                        replica_groups=[list(range(n_cores))],
                        ins=[cc_in.opt()],
                        outs=[cc_out.opt()],
                    )
                    nc.sync.dma_start(sg[:], cc_out[:])
                else:
                    # single-core timing variant (TimelineSim can't model
                    # collectives): mean is just this core's sums
                    nc.vector.tensor_copy(sg[:], s2[:])

                if not do_p3:
                    continue
                # binarize in units of (n, col-range): a small first unit so
                # the serialized out-DMA chain starts as early as possible.
                units = [(0, 0, 900), (0, 900, NPIX)] + [
                    (n, 0, NPIX) for n in range(1, N_PER_CORE)
                ]
                nm = sg[:]
                for n, c0, c1 in units:
                    t0 = (cb * N_PER_CORE + n) * RT
                    ysl = y_sb[:, t0 * TFREE : (t0 + RT) * TFREE]
                    bin_t = opool.tile([128, NPIX], fp8, tag=f"bin{n}", name="bin")
                    w = c1 - c0
                    ae = c0 + w * ACOLS // NPIX
                    de = c0 + w * (ACOLS + DCOLS) // NPIX
                    nc.scalar.activation(
                        bin_t[:, c0:ae], ysl[:, c0:ae], ACT.Sign, bias=nm
                    )
                    nc.vector.tensor_scalar(
                        bin_t[:, ae:de],
                        ysl[:, ae:de],
                        nm,
                        0.0,
                        mybir.AluOpType.add,
                        mybir.AluOpType.is_ge,
                    )
                    nc.gpsimd.tensor_scalar(
                        bin_t[:, de:c1],
                        ysl[:, de:c1],
                        nm,
                        0.0,
                        mybir.AluOpType.add,
                        mybir.AluOpType.is_ge,
                    )
                    nc.sync.dma_start(y_d[n, cb, :, c0:c1], bin_t[:, c0:c1])

    nc.compile()
    return nc


def prep_inputs(x, weight, bias):
    """Host-side shard + layout prep. Returns list of 8 per-core input maps."""
    assert x.shape == (N_TOT, CI, H, W) and x.dtype == np.float32

    # x -> [core, n, p, b, hw]; 3-level fp8 split
    xs = x.reshape(N_CORES, N_PER_CORE, 2, 128, HWF).transpose(0, 1, 3, 2, 4)
    x0 = xs.astype(FP8)
    r1 = xs - x0.astype(np.float32)
    x1 = (r1 * np.float32(LVL_SCALE)).astype(FP8)
    r2 = r1 - x1.astype(np.float32) / np.float32(LVL_SCALE)
    x2 = (r2 * np.float32(LVL_SCALE)).astype(FP8)
    x8 = np.stack([x0, x1, x2], axis=2)  # [core, n, lvl, p, b, hw]
    x8 = np.pad(x8, ((0, 0),) * 5 + ((0, HWPAD - HWF),))

    wb = np.where(weight >= 0, np.float32(1.0), np.float32(-1.0))
    # [cb, co_f, b, ci_p, kh, kw] -> [ci_p, cb, (kh kw), b, co_f]
    w6 = wb.reshape(2, 128, 2, 128, 3, 3)
    w8 = w6.transpose(3, 0, 4, 5, 2, 1).reshape(128, 2, 9 * 2 * 128).astype(FP8)
    w8s = (
        (w6 * np.float32(1.0 / LVL_SCALE))
        .transpose(3, 0, 4, 5, 2, 1)
        .reshape(128, 2, 9 * 2 * 128)
        .astype(FP8)
    )
    return [
        {
            "x8": np.ascontiguousarray(x8[c]),
            "w8": w8,
            "w8s": w8s,
        }
        for c in range(N_CORES)
    ]


def _bin_units():
    """(n, c0, ae, c1): [c0:ae] is Sign-encoded (+-1/0), [ae:c1] is is_ge
    {0,1}.  Mirrors the build()'s binarize unit/engine split."""
    units = [(0, 0, 900), (0, 900, NPIX)] + [
        (n, 0, NPIX) for n in range(1, N_PER_CORE)
    ]
    out = []
    for n, c0, c1 in units:
        w = c1 - c0
        out.append((n, c0, c0 + w * ACOLS // NPIX, c1))
    return out


def gather(results):
    """[{y8: fp8 mixed-encoding}] * 8 -> (32, 256, 54, 54) fp32 +-1."""
    out = np.empty((N_CORES, N_PER_CORE, 2, 128, NPIX), np.float32)
    units = _bin_units()
    for c, r in enumerate(results):
        yf = np.asarray(r["y8"]).view(FP8).astype(np.float32)  # [n, cb, 128, pix]
        for n, c0, ae, c1 in units:
            out[c, n, :, :, c0:ae] = np.where(yf[n, :, :, c0:ae] >= 0, 1.0, -1.0)
            out[c, n, :, :, ae:c1] = yf[n, :, :, ae:c1] * 2.0 - 1.0
    # [core, n, cb, 128, pix] -> [N, 256, 54, 54]
    return out.reshape(N_TOT, CO, OH, OW)


_STATE = {}


def _get_nc():
    if "nc" not in _STATE:
        import concourse.bacc as bacc

        nc = bacc.Bacc(
            "TRN2", target_bir_lowering=False, debug=False, num_devices=N_CORES
        )
        _STATE["nc"] = build(nc)
    return _STATE["nc"]


def kernel(x, weight, bias, _trace=False):
    from concourse.bass_utils import run_bass_kernel_spmd

    nc = _get_nc()
    in_maps = prep_inputs(
        np.asarray(x, np.float32),
        np.asarray(weight, np.float32),
        np.asarray(bias, np.float32),
    )
    res = run_bass_kernel_spmd(
        nc, in_maps, core_ids=list(range(N_CORES)), trace=_trace
    )
    _STATE["last_result"] = res
    return gather(res.results)


# revision 38
# speedup vs baseline: 1.7289x; 1.0071x over previous
"""Binarized 3x3 conv + batchnorm(train) + sign, on 8 TRN2 NeuronCores.

Math: out = sign((y - mean)/sqrt(var+eps)) where y = conv(x, sign(w)) + sign(b)
and mean/var are per-channel batch stats.  Since sqrt(var+eps) > 0, the output
is exactly sign(y - mean_c): variance never needs to be computed.  The +-1
channel bias cancels exactly in sign(y - mean), so it is dropped.

Strategy (data-parallel over batch, 4 images/core):
 - implicit GEMM, all-fp8 DoubleRow: x is split into three e4m3 planes
   x ~= x0 + x1/64 + x2/64 with x0 = e4m3(x), x1 = e4m3((x-x0)*64),
   x2 = e4m3((x-x0-x1/64)*64).  Weights are +-1 (exact in fp8); levels 1,2
   use weights pre-scaled by 2^-6 (exact fp8 normal), so all three levels
   accumulate into a single fp32 PSUM bank: 3 levels x 9 shifts = 27
   DoubleRow matmuls (each contracting both 128-channel input blocks at
   half-cycle/row) per output tile.  ~2x fewer PE cycles than an
   fp16-hi + fp8-lo split, at ~130/23.9M sign flips.
 - rhs of each matmul is a 4D view [128ci, 2b, 9rows, 54cols] so only valid
   output columns are computed (486 of 504).
 - a ~140-matmul warmup chain on scratch data runs during the input-DMA
   lead-in so the PE p-state ramp completes before real matmuls start;
   lead-in DMAs are split/ordered so the first tile's weights + rows arrive
   first (DMA transfers serialize on the DMA engines).
 - drain PSUM -> y_sb (fp32) alternates DVE / ScalarE, harvesting per-channel
   sums via accum_out for free.
 - batch stats are done per output-channel-block (cb): cb=0's sums finish one
   image-pass (~16us) before cb=1's, so cb=0's AllReduce + binarize + store
   hide entirely under cb=1's conv matmuls; only cb=1's tail is exposed.
 - binarize splits each image's columns across ScalarE (Sign -> fp8 +-1/0),
   DVE and Pool (is_ge -> fp8 {0,1}); host maps both encodings to +-1 fp32.
"""

import sys

if "/opt/trn_rl_repo" not in sys.path:
    sys.path.insert(0, "/opt/trn_rl_repo")

import numpy as np
import ml_dtypes

N_CORES = 8
N_PER_CORE = 4          # images per core
CI = 256                # in channels
CO = 256                # out channels
H = W = 56
OH = OW = 54
HWF = H * W             # 3136
SROWS = 29              # input rows per slab (27 owned + 2 overlap)
SLAB = SROWS * W + 8    # 1632; 16B-aligned slab pitch
SOFF = 27 * W           # 1512: slab g starts at input row 27g
NPIX = OH * OW          # 2916
RT = 6                  # row tiles per image (9 rows each)
RROWS = 9
FREE = RROWS * W        # 504 slice length per shift
TFREE = RROWS * OW      # 486 valid outputs per tile
N_TOT = N_CORES * N_PER_CORE
MEAN_SCALE = 1.0 / (N_TOT * NPIX)
NLVL = 3                # fp8 residual levels
LVL_SCALE = 64.0        # per-level residual scale (weights get 1/64, exact fp8)
RTG = 3                 # rt tiles per psum group
NWUP = 110              # p-state warmup matmuls
ACOLS = 815             # binarize column split: ScalarE share
DCOLS = 1546            # DVE share (runs 2x on SBUF operands); Pool gets 555

FP8 = ml_dtypes.float8_e4m3

# binarize work units (n, col-start, col-end) per channel block
UNITS = [(0, 0, 600), (0, 600, NPIX)] + [(n, 0, NPIX) for n in range(1, N_PER_CORE)]


def build(nc, n_cores=N_CORES, stop_after=None):
    """Emit the SPMD program into a bacc.Bacc instance."""
    import concourse.mybir as mybir
    from concourse import tile

    f32 = mybir.dt.float32
    fp8 = mybir.dt.float8e4
    ACT = mybir.ActivationFunctionType
    DR = mybir.MatmulPerfMode.DoubleRow

    x_d = nc.dram_tensor("x8", [N_PER_CORE, NLVL, 128, 2, 2, SLAB], fp8,
                         kind="ExternalInput")
    w_d = nc.dram_tensor("w8", [128, 2, 9 * 2 * 128], fp8, kind="ExternalInput")
    ws_d = nc.dram_tensor("w8s", [128, 2, 9 * 2 * 128], fp8, kind="ExternalInput")
    y_d = nc.dram_tensor("y8", [N_PER_CORE, 2, 128, NPIX], fp8,
                         kind="ExternalOutput")

    n_tiles = N_PER_CORE * 2 * RT  # 48

    with tile.TileContext(nc) as tc:
        with (
            tc.tile_pool(name="wpool", bufs=1) as wpool,
            tc.tile_pool(name="xpool", bufs=2) as xpool,
            tc.tile_pool(name="ypool", bufs=1) as ypool,
            tc.tile_pool(name="spool", bufs=1) as spool,
            tc.tile_pool(name="opool", bufs=4) as opool,
            tc.tile_pool(name="pspool", bufs=7, space="PSUM") as pspool,
            tc.tile_pool(name="wups", bufs=1, space="PSUM") as wups,
            tc.tile_pool(name="drampool", bufs=4, space="DRAM") as drampool,
        ):
            # ---------------- p-state warmup + act-table preload -------------
            wu_w = wpool.tile([128, 2, 128], fp8, name="wu")
            nc.vector.memset(wu_w[:], 1.0)
            scr_in = spool.tile([128, 1], f32, tag="scr0")
            scr_out = spool.tile([128, 1], fp8, tag="scr1")
            nc.vector.memset(scr_in[:], 0.0)
            nc.scalar.activation(scr_out[:], scr_in[:], ACT.Sign)
            wu_ps = wups.tile([128, 128], f32)
            for _ in range(NWUP):
                nc.tensor.matmul(
                    wu_ps[:], wu_w[:], wu_w[:], start=True, stop=True,
                    perf_mode=DR,
                )

            # ---------------- weights + image-0 lead-in DMAs ----------------
            # one serial DMA pipe: order so the first matmuls' deps land first
            w_sb = wpool.tile([128, 2, 9, 2, 128], fp8, name="w8")
            ws_sb = wpool.tile([128, 2, 9, 2, 128], fp8, name="w8s")
            wv = w_d[:].rearrange("p c (s b m) -> p c s b m", s=9, b=2)
            wsv = ws_d[:].rearrange("p c (s b m) -> p c s b m", s=9, b=2)
            x0_sb = [
                xpool.tile([128, 2, 2, SLAB], fp8, tag=f"x{l}", name=f"x{l}")
                for l in range(NLVL)
            ]
            # x is stored as two row-slabs (rows 0-28 / 27-55) so each
            # row-tile-group's matmul reads stay inside one slab: image-0's
            # slab-1 chunks can then arrive after the first matmuls start.
            def xc(l, g, b):
                nc.sync.dma_start(x0_sb[l][:, g, b], x_d[0, l, :, g, b])

            nc.sync.dma_start(w_sb[:, 0], wv[:, 0])
            xc(0, 0, 0)
            xc(0, 0, 1)
            nc.sync.dma_start(ws_sb[:, 0], wsv[:, 0])
            xc(1, 0, 0)
            xc(1, 0, 1)
            xc(2, 0, 0)
            xc(2, 0, 1)
            xc(0, 1, 0)
            xc(0, 1, 1)
            xc(1, 1, 0)
            xc(1, 1, 1)
            nc.sync.dma_start(w_sb[:, 1], wv[:, 1])
            xc(2, 1, 0)
            xc(2, 1, 1)
            nc.sync.dma_start(ws_sb[:, 1], wsv[:, 1])

            y_sb = ypool.tile([128, n_tiles * TFREE], f32)
            sums = spool.tile([128, n_tiles], f32, tag="sums")

            # ---------------- phase 1: conv + drain (+sums) ------------------
            for n in range(N_PER_CORE):
                if n == 0:
                    x_sb = x0_sb
                else:
                    x_sb = [
                        xpool.tile([128, 2, 2, SLAB], fp8, tag=f"x{l}", name=f"x{l}")
                        for l in range(NLVL)
                    ]
                    for l in range(NLVL):
                        nc.sync.dma_start(x_sb[l][:], x_d[n, l])

                for cb in range(2):
                    # last (n, cb)'s groups are [4, 2] tiles so the final
                    # group's two drains run in parallel on DVE + ScalarE
                    groups = (
                        ([0, 1, 2, 3], [4, 5])
                        if (n == N_PER_CORE - 1 and cb == 1)
                        else ([0, 1, 2], [3, 4, 5])
                    )
                    for rts in groups:
                        pss = [
                            pspool.tile([128, TFREE], f32, tag="ps", name=f"ps{i}")
                            for i in range(len(rts))
                        ]
                        for l in range(NLVL):
                            w_l = w_sb if l == 0 else ws_sb
                            for s in range(9):
                                kh, kw = divmod(s, 3)
                                lw = w_l[:, cb, s]
                                for i, rt in enumerate(rts):
                                    g, r = divmod(rt, RTG)
                                    off = (r * RROWS + kh) * W + kw
                                    rhs = (
                                        x_sb[l][:, g, :, off : off + FREE]
                                        .rearrange("p b (r c) -> p b r c", c=W)
                                        [:, :, :, 0:OW]
                                    )
                                    nc.tensor.matmul(
                                        pss[i][:],
                                        lw,
                                        rhs,
                                        start=(l == 0 and s == 0),
                                        stop=(l == NLVL - 1 and s == 8),
                                        perf_mode=DR,
                                    )
                        # drain: y = psum; accum channel sums for free
                        # (Pool/GPSIMD cannot read PSUM, so DVE/ScalarE only)
                        for i, rt in enumerate(rts):
                            t = (cb * N_PER_CORE + n) * RT + rt
                            if t % 2 == 0:
                                nc.scalar.activation(
                                    y_sb[:, t * TFREE : (t + 1) * TFREE],
                                    pss[i][:],
                                    ACT.Copy,
                                    accum_out=sums[:, t : t + 1],
                                )
                            else:
                                nc.vector.tensor_scalar(
                                    y_sb[:, t * TFREE : (t + 1) * TFREE],
                                    pss[i][:],
                                    1.0,
                                    0.0,
                                    mybir.AluOpType.mult,
                                    mybir.AluOpType.add,
                                    accum_out=sums[:, t : t + 1],
                                )

            # ------- phase 2+3 per cb: mean AllReduce, binarize, store -------
            # cb=0's sums are complete one image-pass (~16us) before cb=1's,
            # so its AllReduce + binarize + DMA hide under cb=1's matmuls.
            do_p2 = stop_after not in ("conv",)
            do_p3 = stop_after not in ("conv", "mean")
            for cb in range(2) if do_p2 else ():
                # fused reduce+scale: s2 = sum(-MEAN_SCALE * sums[cb]); the
                # AllReduce is linear so pre-scaling yields -mean directly.
                junk = spool.tile([128, 24], f32, tag=f"junk{cb}")
                s2 = spool.tile([128, 1], f32, tag=f"s2_{cb}")
                nc.vector.tensor_scalar(
                    junk[:],
                    sums[:, cb * 24 : (cb + 1) * 24],
                    -MEAN_SCALE,
                    0.0,
                    mybir.AluOpType.mult,
                    mybir.AluOpType.add,
                    accum_out=s2[:],
                )
                sg = spool.tile([128, 1], f32, tag=f"sg_{cb}")
                if n_cores > 1:
                    cc_in = drampool.tile([128, 1], f32)
                    cc_out = drampool.tile([128, 1], f32)
                    nc.sync.dma_start(cc_in[:], s2[:])
                    nc.gpsimd.collective_compute(
                        "AllReduce",
                        mybir.AluOpType.add,
                        replica_groups=[list(range(n_cores))],
                        ins=[cc_in.opt()],
                        outs=[cc_out.opt()],
                    )
                    nc.sync.dma_start(sg[:], cc_out[:])
                else:
                    # single-core timing variant (TimelineSim can't model
                    # collectives): mean is just this core's sums
                    nc.vector.tensor_copy(sg[:], s2[:])

                if not do_p3:
                    continue
                # binarize in units of (n, col-range): a small first unit so
                # the serialized out-DMA chain starts as early as possible.
                nm = sg[:]
                for n, c0, c1 in UNITS:
                    t0 = (cb * N_PER_CORE + n) * RT
                    ysl = y_sb[:, t0 * TFREE : (t0 + RT) * TFREE]
                    bin_t = opool.tile([128, NPIX], fp8, tag=f"bin{n}", name="bin")
                    w = c1 - c0
                    ae = c0 + w * ACOLS // NPIX
                    de = c0 + w * (ACOLS + DCOLS) // NPIX
                    nc.scalar.activation(
                        bin_t[:, c0:ae], ysl[:, c0:ae], ACT.Sign, bias=nm
                    )
                    nc.vector.tensor_scalar(
                        bin_t[:, ae:de],
                        ysl[:, ae:de],
                        nm,
                        0.0,
                        mybir.AluOpType.add,
                        mybir.AluOpType.is_ge,
                    )
                    nc.gpsimd.tensor_scalar(
                        bin_t[:, de:c1],
                        ysl[:, de:c1],
                        nm,
                        0.0,
                        mybir.AluOpType.add,
                        mybir.AluOpType.is_ge,
                    )
                    nc.sync.dma_start(y_d[n, cb, :, c0:c1], bin_t[:, c0:c1])

    nc.compile()
    return nc


def prep_inputs(x, weight, bias):
    """Host-side shard + layout prep. Returns list of 8 per-core input maps."""
    assert x.shape == (N_TOT, CI, H, W) and x.dtype == np.float32

    # x -> [core, n, p, b, hw]; 3-level fp8 split
    xs = x.reshape(N_CORES, N_PER_CORE, 2, 128, HWF).transpose(0, 1, 3, 2, 4)
    x0 = xs.astype(FP8)
    r1 = xs - x0.astype(np.float32)
    x1 = (r1 * np.float32(LVL_SCALE)).astype(FP8)
    r2 = r1 - x1.astype(np.float32) / np.float32(LVL_SCALE)
    x2 = (r2 * np.float32(LVL_SCALE)).astype(FP8)
    x8 = np.stack([x0, x1, x2], axis=2)  # [core, n, lvl, p, b, hw]
    # -> row slabs [core, n, lvl, p, slab, b, SLAB]
    npx = SROWS * W
    x8 = np.stack(
        [x8[..., g * SOFF : g * SOFF + npx] for g in range(2)], axis=4
    )
    x8 = np.pad(x8, ((0, 0),) * 6 + ((0, SLAB - npx),))

    wb = np.where(weight >= 0, np.float32(1.0), np.float32(-1.0))
    # [cb, co_f, b, ci_p, kh, kw] -> [ci_p, cb, (kh kw), b, co_f]
    w6 = wb.reshape(2, 128, 2, 128, 3, 3)
    w8 = w6.transpose(3, 0, 4, 5, 2, 1).reshape(128, 2, 9 * 2 * 128).astype(FP8)
    w8s = (
        (w6 * np.float32(1.0 / LVL_SCALE))
        .transpose(3, 0, 4, 5, 2, 1)
        .reshape(128, 2, 9 * 2 * 128)
        .astype(FP8)
    )
    return [
        {
            "x8": np.ascontiguousarray(x8[c]),
            "w8": w8,
            "w8s": w8s,
        }
        for c in range(N_CORES)
    ]


def _bin_units():
    """(n, c0, ae, c1): [c0:ae] is Sign-encoded (+-1/0), [ae:c1] is is_ge
    {0,1}.  Mirrors the build()'s binarize unit/engine split."""
    out = []
    for n, c0, c1 in UNITS:
        w = c1 - c0
        out.append((n, c0, c0 + w * ACOLS // NPIX, c1))
    return out


def gather(results):
    """[{y8: fp8 mixed-encoding}] * 8 -> (32, 256, 54, 54) fp32 +-1."""
    out = np.empty((N_CORES, N_PER_CORE, 2, 128, NPIX), np.float32)
    units = _bin_units()
    for c, r in enumerate(results):
        yf = np.asarray(r["y8"]).view(FP8).astype(np.float32)  # [n, cb, 128, pix]
        for n, c0, ae, c1 in units:
            out[c, n, :, :, c0:ae] = np.where(yf[n, :, :, c0:ae] >= 0, 1.0, -1.0)
            out[c, n, :, :, ae:c1] = yf[n, :, :, ae:c1] * 2.0 - 1.0
    # [core, n, cb, 128, pix] -> [N, 256, 54, 54]
    return out.reshape(N_TOT, CO, OH, OW)


_STATE = {}


def _get_nc():
    if "nc" not in _STATE:
        import concourse.bacc as bacc

        nc = bacc.Bacc(
            "TRN2", target_bir_lowering=False, debug=False, num_devices=N_CORES
        )
        _STATE["nc"] = build(nc)
    return _STATE["nc"]


def kernel(x, weight, bias, _trace=False):
    from concourse.bass_utils import run_bass_kernel_spmd

    nc = _get_nc()
    in_maps = prep_inputs(
        np.asarray(x, np.float32),
        np.asarray(weight, np.float32),
        np.asarray(bias, np.float32),
    )
    res = run_bass_kernel_spmd(
        nc, in_maps, core_ids=list(range(N_CORES)), trace=_trace
    )
    _STATE["last_result"] = res
    return gather(res.results)
